# revision 13
# baseline (speedup 1.0000x reference)
"""ActivateAttention Trainium2 kernel — 8 NeuronCores, SPMD, no collectives.

Sharding: core i = (batch b=i//4, head-half hh=(i%4)//2, query-half qh=i%2).
Each core computes 6 heads (3 head-pairs) over 2048 query rows and the full
4096 keys of its batch, producing a PARTIAL output projection over its 384
head-dims; the host sums the two head-half partials per (batch, query-half)
and adds bp.  Weight slices are pre-sliced/pre-transposed on the host
(layout prep only): wq_t/wk_t/wv_t = W[384hh:384hh+384,:].T  [768,384],
wp_t = Wp[:,384hh:384hh+384].T  [384,768], bk_s = bk[384hh:384hh+384].

Per-core pipeline (bf16 compute, f32 PSUM):
  1. weights: DMA f32 -> Pool cast bf16 (host already transposed)
  2. stream q then k/v in 512-row chunks: DMA f32 -> Pool cast bf16 ->
     PE-transpose (psum) -> x^T chunk; proj: qT/kT = W^T.T @ x^T
     (k: +bias, exact GELU on ACT); v natural [keys, h, 64+ones] -> v_aug
  3. attention per (qc 512-query block, head): QK S^T kt-pairs into
     st [128,2,512] psum; ACT exp -> pt bf16 [128,2,512]; PV with 65-col
     v_aug (ones col gives softmax denominators) accumulates xps [65,512];
     normalize via bf16 ones-matmul broadcast + DVE reciprocal -> xT
  4. outproj per qc: out^T partial = xT.T @ wp^T, f32 out (no bias; host)
k/v chunk streaming is emission-interleaved with the first attention pass so
DMA/PE stream work hides under the ACT-bound exp steps.
"""

import numpy as np
from contextlib import ExitStack

from concourse import bass, bacc, mybir, masks, tile
from concourse import bass_utils

F32 = mybir.dt.float32
BF16 = mybir.dt.bfloat16
AF = mybir.ActivationFunctionType
ALU = mybir.AluOpType

B = 2
N = 4096                    # keys per batch
DIM = 768
HDIM = 384                  # head-dims per core (6 heads)
NH = 6                      # heads per core
D = 64
SCALE = D ** -0.5           # 1/8
NQ = 2048                   # query rows per core
N_CORES = 8

NMT = HDIM // 128           # 3 m-tiles (head pairs) per core
NCT = DIM // 128            # 6 contraction tiles
NKC = N // 512              # 8 key/value stream chunks
NQC = NQ // 512             # 4 query blocks per core
NKT = N // 128              # 32 key tiles
NG = NKT // 2               # 16 kt-pairs per pass


def build_nc() -> bass.Bass:
    nc = bacc.Bacc("TRN2", target_bir_lowering=False, debug=False)

    query = nc.declare_dram_parameter("query", [NQ, DIM], F32, False).ap()
    key = nc.declare_dram_parameter("key", [N, DIM], F32, False).ap()
    value = nc.declare_dram_parameter("value", [N, DIM], F32, False).ap()
    wq_t = nc.declare_dram_parameter("wq_t", [DIM, HDIM], F32, False).ap()
    wk_t = nc.declare_dram_parameter("wk_t", [DIM, HDIM], F32, False).ap()
    bk_s = nc.declare_dram_parameter("bk_s", [HDIM], F32, False).ap()
    wv_t = nc.declare_dram_parameter("wv_t", [DIM, HDIM], F32, False).ap()
    wp_t = nc.declare_dram_parameter("wp_t", [HDIM, DIM], F32, False).ap()
    out = nc.declare_dram_parameter("out", [NQ, DIM], F32, True).ap()

    with tile.TileContext(nc) as tc, ExitStack() as ctx:
        # ---------------- persistent tensors ----------------
        cp = ctx.enter_context(tc.tile_pool(name="const", bufs=1))
        ident = cp.tile([128, 128], BF16)
        masks.make_identity(nc, ident[:])
        bk_t = cp.tile([128, NMT], F32)           # bias per partition per mt
        nc.sync.dma_start(out=bk_t[:], in_=bk_s.rearrange("(c p) -> p c", p=128))
        ones16 = cp.tile([1, D], BF16)
        nc.vector.memset(ones16[:], 1.0)

        wq_sb = cp.tile([128, NCT, HDIM], BF16)
        wk_sb = cp.tile([128, NCT, HDIM], BF16)
        wv_sb = cp.tile([128, NCT, HDIM], BF16)
        wp_sb = cp.tile([128, NMT, DIM], BF16)
        qT = cp.tile([128, NMT, NQ], BF16)        # q^T  [pair-dims, n]
        kT = cp.tile([128, NMT, N], BF16)         # gelu(k^T + bk)
        v_aug = cp.tile([128, NKT, NH, D + 1], BF16)  # v natural + ones col
        xT = cp.tile([128, NMT, NQ], BF16)        # attention out, transposed
        nc.vector.memset(v_aug[:, :, :, D:D + 1], 1.0)

        # ---------------- pools ----------------
        wst = ctx.enter_context(tc.tile_pool(name="wst", bufs=2))
        ld = ctx.enter_context(tc.tile_pool(name="ld", bufs=3))
        cast = ctx.enter_context(tc.tile_pool(name="cast", bufs=2))
        xch = ctx.enter_context(tc.tile_pool(name="xch", bufs=2))
        ptp = ctx.enter_context(tc.tile_pool(name="ptp", bufs=3))
        nrm = ctx.enter_context(tc.tile_pool(name="nrm", bufs=2))
        ost = ctx.enter_context(tc.tile_pool(name="ost", bufs=2))
        # PSUM: stp 2x[128,2,512]f32 (4 banks) + xps [65,512]f32 (1) +
        #       pr 2x[128,512]f32 (2) + tp [128,6,128]bf16 (1) = 8 banks
        stp = ctx.enter_context(tc.tile_pool(name="stp", bufs=2, space="PSUM"))
        xaps = ctx.enter_context(tc.tile_pool(name="xaps", bufs=1, space="PSUM"))
        pr = ctx.enter_context(tc.tile_pool(name="pr", bufs=2, space="PSUM"))
        tpp = ctx.enter_context(tc.tile_pool(name="tpp", bufs=1, space="PSUM"))

        # ---------------- weight load (pre-transposed on host) -------------
        def load_w(src, dst, nct):
            for c in range(nct):
                wf = wst.tile([128, dst.shape[2]], F32, tag="wf")
                nc.sync.dma_start(out=wf[:], in_=src[128 * c:128 * (c + 1), :])
                nc.vector.tensor_copy(dst[:, c, :], wf[:])

        load_w(wk_t, wk_sb, NCT)
        load_w(wv_t, wv_sb, NCT)
        load_w(wq_t, wq_sb, NCT)
        load_w(wp_t, wp_sb, NMT)

        # ---------------- input streaming ----------------
        def stream_rowtile(src, row0, xc, t):
            """DMA 128 rows -> cast bf16 (DVE) -> PE transpose -> xc[:,:,128t]."""
            xf = ld.tile([128, DIM], F32, tag="xf")
            nc.sync.dma_start(out=xf[:], in_=src[row0:row0 + 128, :])
            xb = cast.tile([128, DIM], BF16, tag="xb")
            nc.vector.tensor_copy(xb[:], xf[:])
            tp = tpp.tile([128, NCT, 128], BF16, tag="tp")
            for c in range(NCT):
                nc.tensor.transpose(tp[:, c, :], xb[:, 128 * c:128 * (c + 1)],
                                    ident[:])
            nc.vector.tensor_copy(xc[:, :, 128 * t:128 * (t + 1)], tp[:])

        def q_chunk(qc):
            xc = xch.tile([128, NCT, 512], BF16, tag="x")
            for t in range(4):
                stream_rowtile(query, 512 * qc + 128 * t, xc, t)
            for mt in range(NMT):
                pm = pr.tile([128, 512], F32, tag="pr")
                for c in range(NCT):
                    nc.tensor.matmul(pm[:], wq_sb[:, c, 128 * mt:128 * (mt + 1)],
                                     xc[:, c, :], start=(c == 0),
                                     stop=(c == NCT - 1))
                nc.vector.tensor_copy(qT[:, mt, 512 * qc:512 * (qc + 1)], pm[:])

        def kv_chunk(kc):
            xc = xch.tile([128, NCT, 512], BF16, tag="x")
            for t in range(4):
                stream_rowtile(key, 512 * kc + 128 * t, xc, t)
            for mt in range(NMT):
                pm = pr.tile([128, 512], F32, tag="pr")
                for c in range(NCT):
                    nc.tensor.matmul(pm[:], wk_sb[:, c, 128 * mt:128 * (mt + 1)],
                                     xc[:, c, :], start=(c == 0),
                                     stop=(c == NCT - 1))
                nc.scalar.activation(kT[:, mt, 512 * kc:512 * (kc + 1)], pm[:],
                                     AF.Gelu, bias=bk_t[:, mt:mt + 1], scale=1.0)
            vc = xch.tile([128, NCT, 512], BF16, tag="x")
            for t in range(4):
                stream_rowtile(value, 512 * kc + 128 * t, vc, t)
            for t in range(4):
                pv = pr.tile([128, 512], F32, tag="pr")
                for c in range(NCT):
                    nc.tensor.matmul(pv[:, 0:HDIM],
                                     vc[:, c, 128 * t:128 * (t + 1)],
                                     wv_sb[:, c, :], start=(c == 0),
                                     stop=(c == NCT - 1))
                nc.vector.tensor_copy(
                    v_aug[:, 4 * kc + t, :, 0:D],
                    pv[:, 0:HDIM].rearrange("p (h d) -> p h d", d=D))

        # ---------------- attention pass: one head, one 512-query block -----
        # PV is emitted one kt-pair behind exp so the in-order PE queue never
        # blocks on the ACT engine; `fillers` dependency-free transposes per
        # step keep the PE gapless so HAM holds the 2.4 GHz grant.
        def warm_burst(n, nm):
            """Back-to-back same-lhsT matmuls; sustained reload-free array
            streaming is what HAM needs to grant/hold the 2.4 GHz p-state."""
            wt = stp.tile([128, 2, 512], F32, tag="st", name=f"warm{nm}")
            for _ in range(n):
                nc.tensor.matmul(wt[:, 0, :], ident[:], qT[:, 0, 0:512],
                                 start=True, stop=True)

        def attn_pass(h, qc, interleave=None, warm=0, warm_next=0):
            """One head over one 512-query block.  The NEXT pass's warm burst
            is emitted at g==NG, inside the final exp's shadow, so the p-state
            refresh costs no ACT idle at the pass boundary."""
            mt, poff = h // 2, 64 * (h % 2)
            qs = qT[poff:poff + 64, mt, 512 * qc:512 * (qc + 1)]
            if warm:
                warm_burst(warm, f"s{h}_{qc}")
            xps = xaps.tile([65, 512], F32, tag="xa", name=f"xps{h}_{qc}")
            prev_pt = None
            for g in range(NG + 1):
                if interleave is not None and g < NG:
                    interleave(g)
                pt = None
                if g < NG:
                    st = stp.tile([128, 2, 512], F32, tag="st",
                                  name=f"st{h}_{qc}_{g}")
                    for i in range(2):
                        kt = 2 * g + i
                        nc.tensor.matmul(st[:, i, :],
                                         kT[poff:poff + 64, mt,
                                            128 * kt:128 * (kt + 1)],
                                         qs, start=True, stop=True)
                    pt = ptp.tile([128, 2, 512], BF16, tag="pt",
                                  name=f"pt{h}_{qc}_{g}")
                    nc.scalar.activation(pt[:], st[:], AF.Exp, scale=SCALE)
                if g == NG and warm_next:
                    warm_burst(warm_next, f"n{h}_{qc}")
                if prev_pt is not None:
                    gp = g - 1
                    for i in range(2):
                        nc.tensor.matmul(xps[:], v_aug[:, 2 * gp + i, h, :],
                                         prev_pt[:, i, :],
                                         start=(gp == 0 and i == 0),
                                         stop=(gp == NG - 1 and i == 1),
                                         skip_group_check=True)
                prev_pt = pt
            # normalize: broadcast 1/denominator over the 64 dim-partitions
            d16 = nrm.tile([1, 512], BF16, tag="d16")
            nc.vector.tensor_copy(d16[:], xps[64:65, :])
            Rp = stp.tile([128, 2, 512], F32, tag="st", name=f"Rp{h}_{qc}")
            nc.tensor.matmul(Rp[0:64, 0, :], ones16[:], d16[:],
                             start=True, stop=True)
            Rs = nrm.tile([64, 512], F32, tag="Rs")
            nc.vector.reciprocal_approx_fast(Rs[:], Rp[0:64, 0, :])
            nc.vector.tensor_tensor(
                xT[poff:poff + 64, mt, 512 * qc:512 * (qc + 1)],
                xps[0:64, :], Rs[:], op=ALU.mult)

        def outproj(qc):
            for t in range(4):
                r0 = 512 * qc + 128 * t
                ot = ost.tile([128, DIM], F32, tag="ot")
                for o0, w in ((0, 512), (512, 256)):
                    po = pr.tile([128, 512], F32, tag="pr")
                    for c in range(NMT):
                        nc.tensor.matmul(po[:, 0:w],
                                         xT[:, c, r0:r0 + 128],
                                         wp_sb[:, c, o0:o0 + w],
                                         start=(c == 0), stop=(c == NMT - 1))
                    nc.vector.tensor_copy(ot[:, o0:o0 + w], po[:, 0:w])
                nc.sync.dma_start(out=out[r0:r0 + 128, :], in_=ot[:])

        # ---------------- emission schedule ----------------
        q_chunk(0)
        kv_chunk(0)
        kv_chunk(1)

        # pass (h0, qc0) interleaved with remaining k/v chunks: pair g needs
        # chunk g//2, so emit chunk c at step g=2(c-2)
        def kv_ileave(g):
            if g % 2 == 0:
                c = g // 2 + 2
                if c < NKC:
                    kv_chunk(c)

        def q_ileave(qn):
            def f(g):
                if g == 0:
                    q_chunk(qn)
            return f

        for qc in range(NQC):
            for h in range(NH):
                il = None
                if qc == 0 and h == 0:
                    il = kv_ileave
                elif qc == 0 and h in (1, 2, 3):
                    il = q_ileave(h)
                wm = 24 if (qc == 0 and h == 0) else 0
                attn_pass(h, qc, interleave=il, warm=wm, warm_next=6)
            outproj(qc)

    nc.compile()
    return nc


_NC_CACHE = {}


def _get_nc():
    if "nc" not in _NC_CACHE:
        _NC_CACHE["nc"] = build_nc()
    return _NC_CACHE["nc"]


def _core_inputs(query, key, value, Wq, Wk, bk, Wv, Wp):
    """Per-core input dicts: slice batch/query rows and head-dim weights."""
    c = np.ascontiguousarray
    in_maps = []
    for i in range(N_CORES):
        b, hh, qh = i // 4, (i % 4) // 2, i % 2
        sl = slice(HDIM * hh, HDIM * (hh + 1))
        in_maps.append({
            "query": c(query[b, NQ * qh:NQ * (qh + 1), :], dtype=np.float32),
            "key": c(key[b], dtype=np.float32),
            "value": c(value[b], dtype=np.float32),
            "wq_t": c(Wq[sl, :].T, dtype=np.float32),
            "wk_t": c(Wk[sl, :].T, dtype=np.float32),
            "bk_s": c(bk[sl], dtype=np.float32),
            "wv_t": c(Wv[sl, :].T, dtype=np.float32),
            "wp_t": c(Wp[:, sl].T, dtype=np.float32),
        })
    return in_maps


def kernel(query, key, value, Wq, Wk, bk, Wv, Wp, bp, _results_hook=None):
    query = np.asarray(query, dtype=np.float32)
    key = np.asarray(key, dtype=np.float32)
    value = np.asarray(value, dtype=np.float32)
    in_maps = _core_inputs(query, key, value,
                           np.asarray(Wq, dtype=np.float32),
                           np.asarray(Wk, dtype=np.float32),
                           np.asarray(bk, dtype=np.float32),
                           np.asarray(Wv, dtype=np.float32),
                           np.asarray(Wp, dtype=np.float32))
    nc = _get_nc()
    res = bass_utils.run_bass_kernel_spmd(nc, in_maps,
                                          core_ids=list(range(N_CORES)))
    if _results_hook is not None:
        _results_hook(res)

    bp = np.asarray(bp, dtype=np.float32)
    outp = np.empty((B, 2 * NQ, DIM), dtype=np.float32)
    for b in range(B):
        for qh in range(2):
            lo = res.results[b * 4 + qh]["out"]
            hi = res.results[b * 4 + 2 + qh]["out"]
            outp[b, NQ * qh:NQ * (qh + 1), :] = lo + hi + bp
    return outp


# revision 15
# speedup vs baseline: 1.1605x; 1.1605x over previous
"""ActivateAttention Trainium2 kernel — 8 NeuronCores, SPMD, no collectives.

Sharding: core i = (batch b=i//4, head-half hh=(i%4)//2, query-half qh=i%2).
Each core computes 6 heads (3 head-pairs) over 2048 query rows and the full
4096 keys of its batch, producing a PARTIAL output projection over its 384
head-dims; the host sums the two head-half partials per (batch, query-half)
and adds bp.  Weight slices are pre-sliced/pre-transposed on the host
(layout prep only): wq_t/wk_t/wv_t = W[384hh:384hh+384,:].T  [768,384],
wp_t = Wp[:,384hh:384hh+384].T  [384,768], bk_s = bk[384hh:384hh+384].

Per-core pipeline (bf16 compute, f32 PSUM):
  1. weights: DMA f32 -> Pool cast bf16 (host already transposed)
  2. stream q then k/v in 512-row chunks: DMA f32 -> Pool cast bf16 ->
     PE-transpose (psum) -> x^T chunk; proj: qT/kT = W^T.T @ x^T
     (k: +bias, exact GELU on ACT); v natural [keys, h, 64+ones] -> v_aug
  3. attention per (qc 512-query block, head): QK S^T kt-pairs into
     st [128,2,512] psum; ACT exp -> pt bf16 [128,2,512]; PV with 65-col
     v_aug (ones col gives softmax denominators) accumulates xps [65,512];
     normalize via bf16 ones-matmul broadcast + DVE reciprocal -> xT
  4. outproj per qc: out^T partial = xT.T @ wp^T, f32 out (no bias; host)
k/v chunk streaming is emission-interleaved with the first attention pass so
DMA/PE stream work hides under the ACT-bound exp steps.
"""

import numpy as np
from contextlib import ExitStack

from concourse import bass, bacc, mybir, masks, tile
from concourse import bass_utils

F32 = mybir.dt.float32
BF16 = mybir.dt.bfloat16
AF = mybir.ActivationFunctionType
ALU = mybir.AluOpType

B = 2
N = 4096                    # keys per batch
DIM = 768
HDIM = 384                  # head-dims per core (6 heads)
NH = 6                      # heads per core
D = 64
SCALE = D ** -0.5           # 1/8
NQ = 2048                   # query rows per core
N_CORES = 8

NMT = HDIM // 128           # 3 m-tiles (head pairs) per core
NCT = DIM // 128            # 6 contraction tiles
NKC = N // 512              # 8 key/value stream chunks
NQC = NQ // 512             # 4 query blocks per core
NKT = N // 128              # 32 key tiles
NG = NKT // 2               # 16 kt-pairs per pass


def build_nc() -> bass.Bass:
    nc = bacc.Bacc("TRN2", target_bir_lowering=False, debug=False)

    query = nc.declare_dram_parameter("query", [NQ, DIM], F32, False).ap()
    key = nc.declare_dram_parameter("key", [N, DIM], F32, False).ap()
    value = nc.declare_dram_parameter("value", [N, DIM], F32, False).ap()
    wq_t = nc.declare_dram_parameter("wq_t", [DIM, HDIM], F32, False).ap()
    wk_t = nc.declare_dram_parameter("wk_t", [DIM, HDIM], F32, False).ap()
    bk_s = nc.declare_dram_parameter("bk_s", [HDIM], F32, False).ap()
    wv_t = nc.declare_dram_parameter("wv_t", [DIM, HDIM], F32, False).ap()
    wp_t = nc.declare_dram_parameter("wp_t", [HDIM, DIM], F32, False).ap()
    out = nc.declare_dram_parameter("out", [NQ, DIM], F32, True).ap()

    with tile.TileContext(nc) as tc, ExitStack() as ctx:
        # ---------------- persistent tensors ----------------
        cp = ctx.enter_context(tc.tile_pool(name="const", bufs=1))
        ident = cp.tile([128, 128], BF16)
        masks.make_identity(nc, ident[:])
        bk_t = cp.tile([128, NMT], F32)           # bias per partition per mt
        nc.sync.dma_start(out=bk_t[:], in_=bk_s.rearrange("(c p) -> p c", p=128))
        ones16 = cp.tile([1, D], BF16)
        nc.vector.memset(ones16[:], 1.0)

        wq_sb = cp.tile([128, NCT, HDIM], BF16)
        wk_sb = cp.tile([128, NCT, HDIM], BF16)
        wv_sb = cp.tile([128, NCT, HDIM], BF16)
        wp_sb = cp.tile([128, NMT, DIM], BF16)
        qT = cp.tile([128, NMT, NQ], BF16)        # q^T  [pair-dims, n]
        kT = cp.tile([128, NMT, N], BF16)         # gelu(k^T + bk)
        v_aug = cp.tile([128, NKT, NH, D + 1], BF16)  # v natural + ones col
        xT = cp.tile([128, NMT, NQ], BF16)        # attention out, transposed
        nc.vector.memset(v_aug[:, :, :, D:D + 1], 1.0)

        # ---------------- pools ----------------
        wst = ctx.enter_context(tc.tile_pool(name="wst", bufs=2))
        ld = ctx.enter_context(tc.tile_pool(name="ld", bufs=3))
        cast = ctx.enter_context(tc.tile_pool(name="cast", bufs=2))
        xch = ctx.enter_context(tc.tile_pool(name="xch", bufs=2))
        ptp = ctx.enter_context(tc.tile_pool(name="ptp", bufs=3))
        nrm = ctx.enter_context(tc.tile_pool(name="nrm", bufs=2))
        ost = ctx.enter_context(tc.tile_pool(name="ost", bufs=2))
        # PSUM: stp 2x[128,2,512]f32 (4 banks) + xps [65,512]f32 (1) +
        #       pr 2x[128,512]f32 (2) + tp [128,6,128]bf16 (1) = 8 banks
        stp = ctx.enter_context(tc.tile_pool(name="stp", bufs=2, space="PSUM"))
        xaps = ctx.enter_context(tc.tile_pool(name="xaps", bufs=1, space="PSUM"))
        pr = ctx.enter_context(tc.tile_pool(name="pr", bufs=2, space="PSUM"))
        tpp = ctx.enter_context(tc.tile_pool(name="tpp", bufs=1, space="PSUM"))

        # ---------------- weight load (pre-transposed on host) -------------
        def load_w(src, dst, nct):
            for c in range(nct):
                wf = wst.tile([128, dst.shape[2]], F32, tag="wf")
                nc.sync.dma_start(out=wf[:], in_=src[128 * c:128 * (c + 1), :])
                nc.vector.tensor_copy(dst[:, c, :], wf[:])

        load_w(wk_t, wk_sb, NCT)
        load_w(wv_t, wv_sb, NCT)
        load_w(wq_t, wq_sb, NCT)
        load_w(wp_t, wp_sb, NMT)

        # ---------------- input streaming ----------------
        def stream_rowtile(src, row0, xc, t):
            """DMA 128 rows -> cast bf16 (DVE) -> PE transpose -> xc[:,:,128t]."""
            xf = ld.tile([128, DIM], F32, tag="xf")
            nc.sync.dma_start(out=xf[:], in_=src[row0:row0 + 128, :])
            xb = cast.tile([128, DIM], BF16, tag="xb")
            nc.vector.tensor_copy(xb[:], xf[:])
            tp = tpp.tile([128, NCT, 128], BF16, tag="tp")
            for c in range(NCT):
                nc.tensor.transpose(tp[:, c, :], xb[:, 128 * c:128 * (c + 1)],
                                    ident[:])
            nc.vector.tensor_copy(xc[:, :, 128 * t:128 * (t + 1)], tp[:])

        def q_chunk(qc):
            xc = xch.tile([128, NCT, 512], BF16, tag="x")
            for t in range(4):
                stream_rowtile(query, 512 * qc + 128 * t, xc, t)
            for mt in range(NMT):
                pm = pr.tile([128, 512], F32, tag="pr")
                for c in range(NCT):
                    nc.tensor.matmul(pm[:], wq_sb[:, c, 128 * mt:128 * (mt + 1)],
                                     xc[:, c, :], start=(c == 0),
                                     stop=(c == NCT - 1))
                nc.vector.tensor_copy(qT[:, mt, 512 * qc:512 * (qc + 1)], pm[:])

        def kv_chunk(kc):
            xc = xch.tile([128, NCT, 512], BF16, tag="x")
            for t in range(4):
                stream_rowtile(key, 512 * kc + 128 * t, xc, t)
            for mt in range(NMT):
                pm = pr.tile([128, 512], F32, tag="pr")
                for c in range(NCT):
                    nc.tensor.matmul(pm[:], wk_sb[:, c, 128 * mt:128 * (mt + 1)],
                                     xc[:, c, :], start=(c == 0),
                                     stop=(c == NCT - 1))
                nc.scalar.activation(kT[:, mt, 512 * kc:512 * (kc + 1)], pm[:],
                                     AF.Gelu, bias=bk_t[:, mt:mt + 1], scale=1.0)
            vc = xch.tile([128, NCT, 512], BF16, tag="x")
            for t in range(4):
                stream_rowtile(value, 512 * kc + 128 * t, vc, t)
            for t in range(4):
                pv = pr.tile([128, 512], F32, tag="pr")
                for c in range(NCT):
                    nc.tensor.matmul(pv[:, 0:HDIM],
                                     vc[:, c, 128 * t:128 * (t + 1)],
                                     wv_sb[:, c, :], start=(c == 0),
                                     stop=(c == NCT - 1))
                nc.vector.tensor_copy(
                    v_aug[:, 4 * kc + t, :, 0:D],
                    pv[:, 0:HDIM].rearrange("p (h d) -> p h d", d=D))

        # ---------------- attention pass: one head, one 512-query block -----
        # PV is emitted one kt-pair behind exp so the in-order PE queue never
        # blocks on the ACT engine; `fillers` dependency-free transposes per
        # step keep the PE gapless so HAM holds the 2.4 GHz grant.
        def warm_burst(n, nm):
            """Back-to-back same-lhsT matmuls; sustained reload-free array
            streaming is what HAM needs to grant/hold the 2.4 GHz p-state."""
            wt = stp.tile([128, 2, 512], F32, tag="st", name=f"warm{nm}")
            for _ in range(n):
                nc.tensor.matmul(wt[:, 0, :], ident[:], qT[:, 0, 0:512],
                                 start=True, stop=True)

        def attn_pass(h, qc, interleave=None, warm=8, pre=None):
            """One head over one 512-query block.  Returns a normalize
            closure; the caller emits it as the NEXT pass's `pre`, which runs
            right after that pass's warm burst — the d16/Rp reciprocal chain
            then drains under the warm matmuls and the PE never idles at the
            pass boundary."""
            mt, poff = h // 2, 64 * (h % 2)
            qs = qT[poff:poff + 64, mt, 512 * qc:512 * (qc + 1)]
            if warm:
                warm_burst(warm, f"s{h}_{qc}")
            if pre is not None:
                pre()
            xps = xaps.tile([65, 512], F32, tag="xa", name=f"xps{h}_{qc}")
            prev_pt = None
            for g in range(NG + 1):
                if interleave is not None and g < NG:
                    interleave(g)
                pt = None
                if g < NG:
                    st = stp.tile([128, 2, 512], F32, tag="st",
                                  name=f"st{h}_{qc}_{g}")
                    for i in range(2):
                        kt = 2 * g + i
                        nc.tensor.matmul(st[:, i, :],
                                         kT[poff:poff + 64, mt,
                                            128 * kt:128 * (kt + 1)],
                                         qs, start=True, stop=True)
                    pt = ptp.tile([128, 2, 512], BF16, tag="pt",
                                  name=f"pt{h}_{qc}_{g}")
                    nc.scalar.activation(pt[:], st[:], AF.Exp, scale=SCALE)
                if prev_pt is not None:
                    gp = g - 1
                    for i in range(2):
                        nc.tensor.matmul(xps[:], v_aug[:, 2 * gp + i, h, :],
                                         prev_pt[:, i, :],
                                         start=(gp == 0 and i == 0),
                                         stop=(gp == NG - 1 and i == 1),
                                         skip_group_check=True)
                prev_pt = pt

            def normalize():
                # broadcast 1/denominator over the 64 dim-partitions
                d16 = nrm.tile([1, 512], BF16, tag="d16")
                nc.vector.tensor_copy(d16[:], xps[64:65, :])
                Rp = stp.tile([128, 2, 512], F32, tag="st", name=f"Rp{h}_{qc}")
                nc.tensor.matmul(Rp[0:64, 0, :], ones16[:], d16[:],
                                 start=True, stop=True)
                Rs = nrm.tile([64, 512], F32, tag="Rs")
                nc.vector.reciprocal_approx_fast(Rs[:], Rp[0:64, 0, :])
                nc.vector.tensor_tensor(
                    xT[poff:poff + 64, mt, 512 * qc:512 * (qc + 1)],
                    xps[0:64, :], Rs[:], op=ALU.mult)
            return normalize

        def outproj(qc):
            for t in range(4):
                r0 = 512 * qc + 128 * t
                ot = ost.tile([128, DIM], F32, tag="ot")
                for o0, w in ((0, 512), (512, 256)):
                    po = pr.tile([128, 512], F32, tag="pr")
                    for c in range(NMT):
                        nc.tensor.matmul(po[:, 0:w],
                                         xT[:, c, r0:r0 + 128],
                                         wp_sb[:, c, o0:o0 + w],
                                         start=(c == 0), stop=(c == NMT - 1))
                    nc.vector.tensor_copy(ot[:, o0:o0 + w], po[:, 0:w])
                nc.sync.dma_start(out=out[r0:r0 + 128, :], in_=ot[:])

        # ---------------- emission schedule ----------------
        q_chunk(0)
        kv_chunk(0)
        kv_chunk(1)

        # pass (h0, qc0) interleaved with remaining k/v chunks: pair g needs
        # chunk g//2, so emit chunk c at step g=2(c-2)
        def kv_ileave(g):
            if g % 2 == 0:
                c = g // 2 + 2
                if c < NKC:
                    kv_chunk(c)

        def q_ileave(qn):
            def f(g):
                if g == 0:
                    q_chunk(qn)
            return f

        norm = None
        for qc in range(NQC):
            for h in range(NH):
                il = None
                if qc == 0 and h == 0:
                    il = kv_ileave
                elif qc == 0 and h in (1, 2, 3):
                    il = q_ileave(h)
                wm = 24 if (qc == 0 and h == 0) else 8
                norm = attn_pass(h, qc, interleave=il, warm=wm, pre=norm)
                if h == NH - 1:          # h5 normalizes inline: outproj needs it
                    norm()
                    norm = None
            outproj(qc)

    nc.compile()
    return nc


_NC_CACHE = {}


def _get_nc():
    if "nc" not in _NC_CACHE:
        _NC_CACHE["nc"] = build_nc()
    return _NC_CACHE["nc"]


def _core_inputs(query, key, value, Wq, Wk, bk, Wv, Wp):
    """Per-core input dicts: slice batch/query rows and head-dim weights."""
    c = np.ascontiguousarray
    in_maps = []
    for i in range(N_CORES):
        b, hh, qh = i // 4, (i % 4) // 2, i % 2
        sl = slice(HDIM * hh, HDIM * (hh + 1))
        in_maps.append({
            "query": c(query[b, NQ * qh:NQ * (qh + 1), :], dtype=np.float32),
            "key": c(key[b], dtype=np.float32),
            "value": c(value[b], dtype=np.float32),
            "wq_t": c(Wq[sl, :].T, dtype=np.float32),
            "wk_t": c(Wk[sl, :].T, dtype=np.float32),
            "bk_s": c(bk[sl], dtype=np.float32),
            "wv_t": c(Wv[sl, :].T, dtype=np.float32),
            "wp_t": c(Wp[:, sl].T, dtype=np.float32),
        })
    return in_maps


def kernel(query, key, value, Wq, Wk, bk, Wv, Wp, bp, _results_hook=None):
    query = np.asarray(query, dtype=np.float32)
    key = np.asarray(key, dtype=np.float32)
    value = np.asarray(value, dtype=np.float32)
    in_maps = _core_inputs(query, key, value,
                           np.asarray(Wq, dtype=np.float32),
                           np.asarray(Wk, dtype=np.float32),
                           np.asarray(bk, dtype=np.float32),
                           np.asarray(Wv, dtype=np.float32),
                           np.asarray(Wp, dtype=np.float32))
    nc = _get_nc()
    res = bass_utils.run_bass_kernel_spmd(nc, in_maps,
                                          core_ids=list(range(N_CORES)))
    if _results_hook is not None:
        _results_hook(res)

    bp = np.asarray(bp, dtype=np.float32)
    outp = np.empty((B, 2 * NQ, DIM), dtype=np.float32)
    for b in range(B):
        for qh in range(2):
            lo = res.results[b * 4 + qh]["out"]
            hi = res.results[b * 4 + 2 + qh]["out"]
            outp[b, NQ * qh:NQ * (qh + 1), :] = lo + hi + bp
    return outp


# revision 20
# speedup vs baseline: 1.1768x; 1.0141x over previous
"""ActivateAttention Trainium2 kernel — 8 NeuronCores, SPMD, no collectives.

Sharding: core i = (batch b=i//4, head-half hh=(i%4)//2, query-half qh=i%2).
Each core computes 6 heads (3 head-pairs) over 2048 query rows and the full
4096 keys of its batch, producing a PARTIAL output projection over its 384
head-dims; the host sums the two head-half partials per (batch, query-half)
and adds bp.  Weight slices are pre-sliced/pre-transposed on the host
(layout prep only): wq_t/wk_t/wv_t = W[384hh:384hh+384,:].T  [768,384],
wp_t = Wp[:,384hh:384hh+384].T  [384,768], bk_s = bk[384hh:384hh+384].

Per-core pipeline (bf16 compute, f32 PSUM):
  1. weights: DMA f32 -> Pool cast bf16 (host already transposed)
  2. stream q then k/v in 512-row chunks: DMA f32 -> Pool cast bf16 ->
     PE-transpose (psum) -> x^T chunk; proj: qT/kT = W^T.T @ x^T
     (k: +bias, exact GELU on ACT); v natural [keys, h, 64+ones] -> v_aug
  3. attention per (qc 512-query block, head): QK S^T kt-pairs into
     st [128,2,512] psum; ACT exp -> pt bf16 [128,2,512]; PV with 65-col
     v_aug (ones col gives softmax denominators) accumulates xps [65,512];
     normalize via bf16 ones-matmul broadcast + DVE reciprocal -> xT
  4. outproj per qc: out^T partial = xT.T @ wp^T, f32 out (no bias; host)
k/v chunk streaming is emission-interleaved with the first attention pass so
DMA/PE stream work hides under the ACT-bound exp steps.
"""

import numpy as np
from contextlib import ExitStack

from concourse import bass, bacc, mybir, masks, tile
from concourse import bass_utils

F32 = mybir.dt.float32
BF16 = mybir.dt.bfloat16
AF = mybir.ActivationFunctionType
ALU = mybir.AluOpType

B = 2
N = 4096                    # keys per batch
DIM = 768
HDIM = 384                  # head-dims per core (6 heads)
NH = 6                      # heads per core
D = 64
SCALE = D ** -0.5           # 1/8
NQ = 2048                   # query rows per core
N_CORES = 8

NMT = HDIM // 128           # 3 m-tiles (head pairs) per core
NCT = DIM // 128            # 6 contraction tiles
NKC = N // 512              # 8 key/value stream chunks
NQC = NQ // 512             # 4 query blocks per core
NKT = N // 128              # 32 key tiles
NG = NKT // 2               # 16 kt-pairs per pass


def build_nc() -> bass.Bass:
    nc = bacc.Bacc("TRN2", target_bir_lowering=False, debug=False)

    query = nc.declare_dram_parameter("query", [NQ, DIM], F32, False).ap()
    key = nc.declare_dram_parameter("key", [N, DIM], F32, False).ap()
    value = nc.declare_dram_parameter("value", [N, DIM], F32, False).ap()
    wq_t = nc.declare_dram_parameter("wq_t", [DIM, HDIM], F32, False).ap()
    wk_t = nc.declare_dram_parameter("wk_t", [DIM, HDIM], F32, False).ap()
    bk_s = nc.declare_dram_parameter("bk_s", [HDIM], F32, False).ap()
    wv_t = nc.declare_dram_parameter("wv_t", [DIM, HDIM], F32, False).ap()
    wp_t = nc.declare_dram_parameter("wp_t", [HDIM, DIM], F32, False).ap()
    out = nc.declare_dram_parameter("out", [NQ, DIM], F32, True).ap()

    with tile.TileContext(nc) as tc, ExitStack() as ctx:
        # ---------------- persistent tensors ----------------
        cp = ctx.enter_context(tc.tile_pool(name="const", bufs=1))
        ident = cp.tile([128, 128], BF16)
        masks.make_identity(nc, ident[:])
        bk_t = cp.tile([128, NMT], F32)           # bias per partition per mt
        nc.sync.dma_start(out=bk_t[:], in_=bk_s.rearrange("(c p) -> p c", p=128))
        ones16 = cp.tile([1, D], BF16)
        nc.vector.memset(ones16[:], 1.0)

        wq_sb = cp.tile([128, NCT, HDIM], BF16)
        wk_sb = cp.tile([128, NCT, HDIM], BF16)
        wv_sb = cp.tile([128, NCT, HDIM], BF16)
        wp_sb = cp.tile([128, NMT, DIM], BF16)
        qT = cp.tile([128, NMT, NQ], BF16)        # q^T  [pair-dims, n]
        kT = cp.tile([128, NMT, N], BF16)         # gelu(k^T + bk)
        v_aug = cp.tile([128, NKT, NH, D + 1], BF16)  # v natural + ones col
        xT = cp.tile([128, NMT, NQ], BF16)        # attention out, transposed
        nc.vector.memset(v_aug[:, :, :, D:D + 1], 1.0)

        # ---------------- pools ----------------
        wst = ctx.enter_context(tc.tile_pool(name="wst", bufs=2))
        ld = ctx.enter_context(tc.tile_pool(name="ld", bufs=3))
        cast = ctx.enter_context(tc.tile_pool(name="cast", bufs=2))
        xch = ctx.enter_context(tc.tile_pool(name="xch", bufs=2))
        ptp = ctx.enter_context(tc.tile_pool(name="ptp", bufs=3))
        nrm = ctx.enter_context(tc.tile_pool(name="nrm", bufs=2))
        ost = ctx.enter_context(tc.tile_pool(name="ost", bufs=2))
        # PSUM: stp 2x[128,2,512]f32 (4 banks) + xps [65,512]f32 (1) +
        #       pr 2x[128,512]f32 (2) + tp [128,6,128]bf16 (1) = 8 banks
        stp = ctx.enter_context(tc.tile_pool(name="stp", bufs=2, space="PSUM"))
        xaps = ctx.enter_context(tc.tile_pool(name="xaps", bufs=1, space="PSUM"))
        pr = ctx.enter_context(tc.tile_pool(name="pr", bufs=2, space="PSUM"))
        tpp = ctx.enter_context(tc.tile_pool(name="tpp", bufs=1, space="PSUM"))

        # ---------------- weight load (pre-transposed on host) -------------
        def load_w(src, dst, nct):
            for c in range(nct):
                wf = wst.tile([128, dst.shape[2]], F32, tag="wf")
                nc.sync.dma_start(out=wf[:], in_=src[128 * c:128 * (c + 1), :])
                nc.vector.tensor_copy(dst[:, c, :], wf[:])

        load_w(wk_t, wk_sb, NCT)    # wk first: K gelu gates the first pass

        # ---------------- input streaming ----------------
        def stream_rowtile(src, row0, xc, t):
            """DMA 128 rows -> cast bf16 (DVE) -> PE transpose -> xc[:,:,128t]."""
            xf = ld.tile([128, DIM], F32, tag="xf")
            nc.sync.dma_start(out=xf[:], in_=src[row0:row0 + 128, :])
            xb = cast.tile([128, DIM], BF16, tag="xb")
            nc.vector.tensor_copy(xb[:], xf[:])
            tp = tpp.tile([128, NCT, 128], BF16, tag="tp")
            for c in range(NCT):
                nc.tensor.transpose(tp[:, c, :], xb[:, 128 * c:128 * (c + 1)],
                                    ident[:])
            nc.vector.tensor_copy(xc[:, :, 128 * t:128 * (t + 1)], tp[:])

        def q_chunk(qc):
            xc = xch.tile([128, NCT, 512], BF16, tag="x")
            for t in range(4):
                stream_rowtile(query, 512 * qc + 128 * t, xc, t)
            for mt in range(NMT):
                pm = pr.tile([128, 512], F32, tag="pr")
                for c in range(NCT):
                    nc.tensor.matmul(pm[:], wq_sb[:, c, 128 * mt:128 * (mt + 1)],
                                     xc[:, c, :], start=(c == 0),
                                     stop=(c == NCT - 1))
                nc.vector.tensor_copy(qT[:, mt, 512 * qc:512 * (qc + 1)], pm[:])

        def kv_chunk_k(kc):
            xc = xch.tile([128, NCT, 512], BF16, tag="x")
            for t in range(4):
                stream_rowtile(key, 512 * kc + 128 * t, xc, t)
            for mt in range(NMT):
                pm = pr.tile([128, 512], F32, tag="pr")
                for c in range(NCT):
                    nc.tensor.matmul(pm[:], wk_sb[:, c, 128 * mt:128 * (mt + 1)],
                                     xc[:, c, :], start=(c == 0),
                                     stop=(c == NCT - 1))
                nc.scalar.activation(kT[:, mt, 512 * kc:512 * (kc + 1)], pm[:],
                                     AF.Gelu, bias=bk_t[:, mt:mt + 1], scale=1.0)

        def kv_chunk_v(kc):
            vc = xch.tile([128, NCT, 512], BF16, tag="x")
            for t in range(4):
                stream_rowtile(value, 512 * kc + 128 * t, vc, t)
            for t in range(4):
                pv = pr.tile([128, 512], F32, tag="pr")
                for c in range(NCT):
                    nc.tensor.matmul(pv[:, 0:HDIM],
                                     vc[:, c, 128 * t:128 * (t + 1)],
                                     wv_sb[:, c, :], start=(c == 0),
                                     stop=(c == NCT - 1))
                nc.vector.tensor_copy(
                    v_aug[:, 4 * kc + t, :, 0:D],
                    pv[:, 0:HDIM].rearrange("p (h d) -> p h d", d=D))

        def kv_chunk(kc):
            kv_chunk_k(kc)
            kv_chunk_v(kc)

        # ---------------- attention pass: one head, one 512-query block -----
        # PV is emitted one kt-pair behind exp so the in-order PE queue never
        # blocks on the ACT engine; `fillers` dependency-free transposes per
        # step keep the PE gapless so HAM holds the 2.4 GHz grant.
        def warm_burst(n, nm):
            """Back-to-back same-lhsT matmuls; sustained reload-free array
            streaming is what HAM needs to grant/hold the 2.4 GHz p-state."""
            wt = stp.tile([128, 2, 512], F32, tag="st", name=f"warm{nm}")
            for _ in range(n):
                nc.tensor.matmul(wt[:, 0, :], ident[:], qT[:, 0, 0:512],
                                 start=True, stop=True)

        def attn_pass(h, qc, interleave=None, warm=8, pre=None):
            """One head over one 512-query block.  Returns a normalize
            closure; the caller emits it as the NEXT pass's `pre`, which runs
            right after that pass's warm burst — the d16/Rp reciprocal chain
            then drains under the warm matmuls and the PE never idles at the
            pass boundary."""
            mt, poff = h // 2, 64 * (h % 2)
            qs = qT[poff:poff + 64, mt, 512 * qc:512 * (qc + 1)]
            if warm:
                warm_burst(warm, f"s{h}_{qc}")
            if pre is not None:
                pre()
            xps = xaps.tile([65, 512], F32, tag="xa", name=f"xps{h}_{qc}")
            prev_pt = None
            for g in range(NG + 1):
                pt = None
                if g < NG:
                    st = stp.tile([128, 2, 512], F32, tag="st",
                                  name=f"st{h}_{qc}_{g}")
                    for i in range(2):
                        kt = 2 * g + i
                        nc.tensor.matmul(st[:, i, :],
                                         kT[poff:poff + 64, mt,
                                            128 * kt:128 * (kt + 1)],
                                         qs, start=True, stop=True)
                    pt = ptp.tile([128, 2, 512], BF16, tag="pt",
                                  name=f"pt{h}_{qc}_{g}")
                    nc.scalar.activation(pt[:], st[:], AF.Exp, scale=SCALE)
                # interleaved work goes AFTER the QK/exp emission so the ACT
                # engine always has its next exp queued before PE bulk work
                if interleave is not None and g < NG:
                    interleave(g)
                if prev_pt is not None:
                    gp = g - 1
                    for i in range(2):
                        nc.tensor.matmul(xps[:], v_aug[:, 2 * gp + i, h, :],
                                         prev_pt[:, i, :],
                                         start=(gp == 0 and i == 0),
                                         stop=(gp == NG - 1 and i == 1),
                                         skip_group_check=True)
                prev_pt = pt

            def normalize():
                # broadcast 1/denominator over the 64 dim-partitions
                d16 = nrm.tile([1, 512], BF16, tag="d16")
                nc.vector.tensor_copy(d16[:], xps[64:65, :])
                Rp = stp.tile([128, 2, 512], F32, tag="st", name=f"Rp{h}_{qc}")
                nc.tensor.matmul(Rp[0:64, 0, :], ones16[:], d16[:],
                                 start=True, stop=True)
                Rs = nrm.tile([64, 512], F32, tag="Rs")
                nc.vector.reciprocal_approx_fast(Rs[:], Rp[0:64, 0, :])
                nc.vector.tensor_tensor(
                    xT[poff:poff + 64, mt, 512 * qc:512 * (qc + 1)],
                    xps[0:64, :], Rs[:], op=ALU.mult)
            return normalize

        def outproj_parts(qc):
            """Output projection for one qc as 8 thunks (4 n-tiles x 2 column
            halves) so it can be drip-fed into a later pass's PE slack."""
            state = {}
            thunks = []
            for t in range(4):
                for o0, w in ((0, 512), (512, 256)):
                    def f(t=t, o0=o0, w=w):
                        r0 = 512 * qc + 128 * t
                        if o0 == 0:
                            state[t] = ost.tile([128, DIM], F32, tag="ot",
                                                name=f"ot{qc}_{t}")
                        ot = state[t]
                        po = pr.tile([128, 512], F32, tag="pr")
                        for c in range(NMT):
                            nc.tensor.matmul(po[:, 0:w],
                                             xT[:, c, r0:r0 + 128],
                                             wp_sb[:, c, o0:o0 + w],
                                             start=(c == 0),
                                             stop=(c == NMT - 1))
                        nc.vector.tensor_copy(ot[:, o0:o0 + w], po[:, 0:w])
                        if o0 != 0:
                            nc.sync.dma_start(out=out[r0:r0 + 128, :],
                                              in_=ot[:])
                    thunks.append(f)
            return thunks

        # ---------------- emission schedule ----------------
        kv_chunk_k(0)
        load_w(wq_t, wq_sb, NCT)
        q_chunk(0)
        load_w(wv_t, wv_sb, NCT)
        kv_chunk_v(0)
        kv_chunk(1)
        load_w(wp_t, wp_sb, NMT)

        # pass (h0, qc0) interleaved with remaining k/v chunks: pair g needs
        # chunk g//2, so emit chunk c at step g=2(c-2)
        def kv_ileave(g):
            if g % 2 == 0:
                c = g // 2 + 2
                if c < NKC:
                    kv_chunk(c)

        def q_ileave(qn):
            def f(g):
                if g == 0:
                    q_chunk(qn)
            return f

        def op_ileave(thunks):
            def f(g):
                if g < len(thunks):
                    thunks[g]()
            return f

        norm = None
        for qc in range(NQC):
            for h in range(NH):
                il = None
                if qc == 0 and h == 0:
                    il = kv_ileave
                elif qc == 0 and h in (1, 2, 3):
                    il = q_ileave(h)
                elif h == 0 and qc > 0:
                    il = op_ileave(outproj_parts(qc - 1))
                wm = 24 if (qc == 0 and h == 0) else 8
                norm = attn_pass(h, qc, interleave=il, warm=wm, pre=norm)
        norm()
        for f in outproj_parts(NQC - 1):
            f()

    nc.compile()
    return nc


_NC_CACHE = {}


def _get_nc():
    if "nc" not in _NC_CACHE:
        _NC_CACHE["nc"] = build_nc()
    return _NC_CACHE["nc"]


def _core_inputs(query, key, value, Wq, Wk, bk, Wv, Wp):
    """Per-core input dicts: slice batch/query rows and head-dim weights."""
    c = np.ascontiguousarray
    in_maps = []
    for i in range(N_CORES):
        b, hh, qh = i // 4, (i % 4) // 2, i % 2
        sl = slice(HDIM * hh, HDIM * (hh + 1))
        in_maps.append({
            "query": c(query[b, NQ * qh:NQ * (qh + 1), :], dtype=np.float32),
            "key": c(key[b], dtype=np.float32),
            "value": c(value[b], dtype=np.float32),
            "wq_t": c(Wq[sl, :].T, dtype=np.float32),
            "wk_t": c(Wk[sl, :].T, dtype=np.float32),
            "bk_s": c(bk[sl], dtype=np.float32),
            "wv_t": c(Wv[sl, :].T, dtype=np.float32),
            "wp_t": c(Wp[:, sl].T, dtype=np.float32),
        })
    return in_maps


def kernel(query, key, value, Wq, Wk, bk, Wv, Wp, bp, _results_hook=None):
    query = np.asarray(query, dtype=np.float32)
    key = np.asarray(key, dtype=np.float32)
    value = np.asarray(value, dtype=np.float32)
    in_maps = _core_inputs(query, key, value,
                           np.asarray(Wq, dtype=np.float32),
                           np.asarray(Wk, dtype=np.float32),
                           np.asarray(bk, dtype=np.float32),
                           np.asarray(Wv, dtype=np.float32),
                           np.asarray(Wp, dtype=np.float32))
    nc = _get_nc()
    res = bass_utils.run_bass_kernel_spmd(nc, in_maps,
                                          core_ids=list(range(N_CORES)))
    if _results_hook is not None:
        _results_hook(res)

    bp = np.asarray(bp, dtype=np.float32)
    outp = np.empty((B, 2 * NQ, DIM), dtype=np.float32)
    for b in range(B):
        for qh in range(2):
            lo = res.results[b * 4 + qh]["out"]
            hi = res.results[b * 4 + 2 + qh]["out"]
            outp[b, NQ * qh:NQ * (qh + 1), :] = lo + hi + bp
    return outp


# revision 26
# speedup vs baseline: 1.2189x; 1.0358x over previous
"""ActivateAttention Trainium2 kernel — 8 NeuronCores, SPMD, no collectives.

Sharding: core i = (batch b=i//4, head-half hh=(i%4)//2, query-half qh=i%2).
Each core computes 6 heads (3 head-pairs) over 2048 query rows and the full
4096 keys of its batch, producing a PARTIAL output projection over its 384
head-dims; the host sums the two head-half partials per (batch, query-half)
and adds bp.  Weight slices are pre-sliced/pre-transposed on the host
(layout prep only): wq_t/wk_t/wv_t = W[384hh:384hh+384,:].T  [768,384],
wp_t = Wp[:,384hh:384hh+384].T  [384,768], bk_s = bk[384hh:384hh+384].

Per-core pipeline (bf16 compute, f32 PSUM):
  1. weights: DMA f32 -> Pool cast bf16 (host already transposed)
  2. stream q then k/v in 512-row chunks: DMA f32 -> Pool cast bf16 ->
     PE-transpose (psum) -> x^T chunk; proj: qT/kT = W^T.T @ x^T
     (k: +bias, exact GELU on ACT); v natural [keys, h, 64+ones] -> v_aug
  3. attention per (qc 512-query block, head): QK S^T kt-pairs into
     st [128,2,512] psum; ACT exp -> pt bf16 [128,2,512]; PV with 65-col
     v_aug (ones col gives softmax denominators) accumulates xps [65,512];
     normalize via bf16 ones-matmul broadcast + DVE reciprocal -> xT
  4. outproj per qc: out^T partial = xT.T @ wp^T, f32 out (no bias; host)
k/v chunk streaming is emission-interleaved with the first attention pass so
DMA/PE stream work hides under the ACT-bound exp steps.
"""

import numpy as np
from contextlib import ExitStack

from concourse import bass, bacc, mybir, masks, tile
from concourse import bass_utils

F32 = mybir.dt.float32
BF16 = mybir.dt.bfloat16
AF = mybir.ActivationFunctionType
ALU = mybir.AluOpType

B = 2
N = 4096                    # keys per batch
DIM = 768
HDIM = 384                  # head-dims per core (6 heads)
NH = 6                      # heads per core
D = 64
SCALE = D ** -0.5           # 1/8
NQ = 2048                   # query rows per core
N_CORES = 8

NMT = HDIM // 128           # 3 m-tiles (head pairs) per core
NCT = DIM // 128            # 6 contraction tiles
NKC = N // 512              # 8 key/value stream chunks
NQC = NQ // 512             # 4 query blocks per core
NKT = N // 128              # 32 key tiles
NG = NKT // 2               # 16 kt-pairs per pass

# Schraudolph exp constants: bitcast(int32(S*SCH_A + SCH_B)) ~ exp(S*SCALE)
SCH_A = float(np.float32(12102203.161561485 * SCALE))   # 2^23/ln2 * SCALE
SCH_B = 1064866805.0


def build_nc() -> bass.Bass:
    nc = bacc.Bacc("TRN2", target_bir_lowering=False, debug=False)

    query = nc.declare_dram_parameter("query", [NQ, DIM], F32, False).ap()
    key = nc.declare_dram_parameter("key", [N, DIM], F32, False).ap()
    value = nc.declare_dram_parameter("value", [N, DIM], F32, False).ap()
    wq_t = nc.declare_dram_parameter("wq_t", [DIM, HDIM], F32, False).ap()
    wk_t = nc.declare_dram_parameter("wk_t", [DIM, HDIM], F32, False).ap()
    bk_s = nc.declare_dram_parameter("bk_s", [HDIM], F32, False).ap()
    wv_t = nc.declare_dram_parameter("wv_t", [DIM, HDIM], F32, False).ap()
    wp_t = nc.declare_dram_parameter("wp_t", [HDIM, DIM], F32, False).ap()
    out = nc.declare_dram_parameter("out", [NQ, DIM], F32, True).ap()

    with tile.TileContext(nc) as tc, ExitStack() as ctx:
        # ---------------- persistent tensors ----------------
        cp = ctx.enter_context(tc.tile_pool(name="const", bufs=1))
        ident = cp.tile([128, 128], BF16)
        masks.make_identity(nc, ident[:])
        bk_t = cp.tile([128, NMT], F32)           # bias per partition per mt
        nc.sync.dma_start(out=bk_t[:], in_=bk_s.rearrange("(c p) -> p c", p=128))
        ones16 = cp.tile([1, D], BF16)
        nc.vector.memset(ones16[:], 1.0)

        wq_sb = cp.tile([128, NCT, HDIM], BF16)
        wk_sb = cp.tile([128, NCT, HDIM], BF16)
        wv_sb = cp.tile([128, NCT, HDIM], BF16)
        wp_sb = cp.tile([128, NMT, DIM], BF16)
        qT = cp.tile([128, NMT, NQ], BF16)        # q^T  [pair-dims, n]
        kT = cp.tile([128, NMT, N], BF16)         # gelu(k^T + bk)
        v_aug = cp.tile([128, NKT, NH, D + 1], BF16)  # v natural + ones col
        xT = cp.tile([128, NMT, NQ], BF16)        # attention out, transposed
        nc.vector.memset(v_aug[:, :, :, D:D + 1], 1.0)

        # ---------------- pools ----------------
        wst = ctx.enter_context(tc.tile_pool(name="wst", bufs=2))
        schp = ctx.enter_context(tc.tile_pool(name="schp", bufs=2))
        ld = ctx.enter_context(tc.tile_pool(name="ld", bufs=3))
        cast = ctx.enter_context(tc.tile_pool(name="cast", bufs=2))
        xch = ctx.enter_context(tc.tile_pool(name="xch", bufs=2))
        ptp = ctx.enter_context(tc.tile_pool(name="ptp", bufs=3))
        nrm = ctx.enter_context(tc.tile_pool(name="nrm", bufs=2))
        ost = ctx.enter_context(tc.tile_pool(name="ost", bufs=2))
        # PSUM: stp 2x[128,2,512]f32 (4 banks) + xps [65,512]f32 (1) +
        #       pr 2x[128,512]f32 (2) + tp [128,6,128]bf16 (1) = 8 banks
        stp = ctx.enter_context(tc.tile_pool(name="stp", bufs=2, space="PSUM"))
        xaps = ctx.enter_context(tc.tile_pool(name="xaps", bufs=1, space="PSUM"))
        pr = ctx.enter_context(tc.tile_pool(name="pr", bufs=2, space="PSUM"))
        tpp = ctx.enter_context(tc.tile_pool(name="tpp", bufs=1, space="PSUM"))

        # ---------------- weight load (pre-transposed on host) -------------
        def load_w(src, dst, nct):
            for c in range(nct):
                wf = wst.tile([128, dst.shape[2]], F32, tag="wf")
                nc.sync.dma_start(out=wf[:], in_=src[128 * c:128 * (c + 1), :])
                nc.vector.tensor_copy(dst[:, c, :], wf[:])

        load_w(wk_t, wk_sb, NCT)    # wk first: K gelu gates the first pass

        # ---------------- input streaming ----------------
        def stream_rowtile(src, row0, xc, t):
            """DMA 128 rows -> cast bf16 (DVE) -> PE transpose -> xc[:,:,128t]."""
            xf = ld.tile([128, DIM], F32, tag="xf")
            nc.sync.dma_start(out=xf[:], in_=src[row0:row0 + 128, :])
            xb = cast.tile([128, DIM], BF16, tag="xb")
            nc.vector.tensor_copy(xb[:], xf[:])
            tp = tpp.tile([128, NCT, 128], BF16, tag="tp")
            for c in range(NCT):
                nc.tensor.transpose(tp[:, c, :], xb[:, 128 * c:128 * (c + 1)],
                                    ident[:])
            nc.vector.tensor_copy(xc[:, :, 128 * t:128 * (t + 1)], tp[:])

        def q_chunk(qc):
            xc = xch.tile([128, NCT, 512], BF16, tag="x")
            for t in range(4):
                stream_rowtile(query, 512 * qc + 128 * t, xc, t)
            for mt in range(NMT):
                pm = pr.tile([128, 512], F32, tag="pr")
                for c in range(NCT):
                    nc.tensor.matmul(pm[:], wq_sb[:, c, 128 * mt:128 * (mt + 1)],
                                     xc[:, c, :], start=(c == 0),
                                     stop=(c == NCT - 1))
                nc.vector.tensor_copy(qT[:, mt, 512 * qc:512 * (qc + 1)], pm[:])

        def kv_chunk_k(kc):
            xc = xch.tile([128, NCT, 512], BF16, tag="x")
            for t in range(4):
                stream_rowtile(key, 512 * kc + 128 * t, xc, t)
            for mt in range(NMT):
                pm = pr.tile([128, 512], F32, tag="pr")
                for c in range(NCT):
                    nc.tensor.matmul(pm[:], wk_sb[:, c, 128 * mt:128 * (mt + 1)],
                                     xc[:, c, :], start=(c == 0),
                                     stop=(c == NCT - 1))
                nc.scalar.activation(kT[:, mt, 512 * kc:512 * (kc + 1)], pm[:],
                                     AF.Gelu, bias=bk_t[:, mt:mt + 1], scale=1.0)

        def kv_chunk_v(kc):
            vc = xch.tile([128, NCT, 512], BF16, tag="x")
            for t in range(4):
                stream_rowtile(value, 512 * kc + 128 * t, vc, t)
            for t in range(4):
                pv = pr.tile([128, 512], F32, tag="pr")
                for c in range(NCT):
                    nc.tensor.matmul(pv[:, 0:HDIM],
                                     vc[:, c, 128 * t:128 * (t + 1)],
                                     wv_sb[:, c, :], start=(c == 0),
                                     stop=(c == NCT - 1))
                nc.vector.tensor_copy(
                    v_aug[:, 4 * kc + t, :, 0:D],
                    pv[:, 0:HDIM].rearrange("p (h d) -> p h d", d=D))

        def kv_chunk(kc):
            kv_chunk_k(kc)
            kv_chunk_v(kc)

        # ---------------- attention pass: one head, one 512-query block -----
        # PV is emitted one kt-pair behind exp so the in-order PE queue never
        # blocks on the ACT engine; `fillers` dependency-free transposes per
        # step keep the PE gapless so HAM holds the 2.4 GHz grant.
        def warm_burst(n, nm):
            """Back-to-back same-lhsT matmuls; sustained reload-free array
            streaming is what HAM needs to grant/hold the 2.4 GHz p-state."""
            wt = stp.tile([128, 2, 512], F32, tag="st", name=f"warm{nm}")
            for _ in range(n):
                nc.tensor.matmul(wt[:, 0, :], ident[:], qT[:, 0, 0:512],
                                 start=True, stop=True)

        def attn_pass(h, qc, interleave=None, warm=8, pre=None,
                      sch_steps=()):
            """One head over one 512-query block.  Returns a normalize
            closure; the caller emits it as the NEXT pass's `pre`, which runs
            right after that pass's warm burst — the d16/Rp reciprocal chain
            then drains under the warm matmuls and the PE never idles at the
            pass boundary."""
            mt, poff = h // 2, 64 * (h % 2)
            qs = qT[poff:poff + 64, mt, 512 * qc:512 * (qc + 1)]
            if warm:
                warm_burst(warm, f"s{h}_{qc}")
            if pre is not None:
                pre()
            xps = xaps.tile([65, 512], F32, tag="xa", name=f"xps{h}_{qc}")
            prev_pt = None
            for g in range(NG + 1):
                pt = None
                if g < NG:
                    st = stp.tile([128, 2, 512], F32, tag="st",
                                  name=f"st{h}_{qc}_{g}")
                    for i in range(2):
                        kt = 2 * g + i
                        nc.tensor.matmul(st[:, i, :],
                                         kT[poff:poff + 64, mt,
                                            128 * kt:128 * (kt + 1)],
                                         qs, start=True, stop=True)
                    pt = ptp.tile([128, 2, 512], BF16, tag="pt",
                                  name=f"pt{h}_{qc}_{g}")
                    if g in sch_steps:
                        # Schraudolph exp on DVE: bitcast(int32(S*A + B)) ~
                        # e^(S*SCALE) to ~4% — offloads ACT, the softmax wall
                        i32 = schp.tile([128, 2, 512], mybir.dt.int32,
                                        tag="i32", name=f"i32{h}_{qc}_{g}")
                        nc.vector.tensor_scalar(i32[:], st[:], SCH_A, SCH_B,
                                                ALU.mult, ALU.add)
                        nc.vector.tensor_copy(pt[:], i32[:].bitcast(F32))
                    else:
                        nc.scalar.activation(pt[:], st[:], AF.Exp, scale=SCALE)
                # interleaved work goes AFTER the QK/exp emission so the ACT
                # engine always has its next exp queued before PE bulk work
                if interleave is not None and g < NG:
                    interleave(g)
                if prev_pt is not None:
                    gp = g - 1
                    for i in range(2):
                        nc.tensor.matmul(xps[:], v_aug[:, 2 * gp + i, h, :],
                                         prev_pt[:, i, :],
                                         start=(gp == 0 and i == 0),
                                         stop=(gp == NG - 1 and i == 1),
                                         skip_group_check=True)
                prev_pt = pt

            def normalize():
                # broadcast 1/denominator over the 64 dim-partitions
                d16 = nrm.tile([1, 512], BF16, tag="d16")
                nc.vector.tensor_copy(d16[:], xps[64:65, :])
                Rp = stp.tile([128, 2, 512], F32, tag="st", name=f"Rp{h}_{qc}")
                nc.tensor.matmul(Rp[0:64, 0, :], ones16[:], d16[:],
                                 start=True, stop=True)
                Rs = nrm.tile([64, 512], F32, tag="Rs")
                nc.vector.reciprocal_approx_fast(Rs[:], Rp[0:64, 0, :])
                nc.vector.tensor_tensor(
                    xT[poff:poff + 64, mt, 512 * qc:512 * (qc + 1)],
                    xps[0:64, :], Rs[:], op=ALU.mult)
            return normalize

        def outproj_parts(qc):
            """Output projection for one qc as 8 thunks (4 n-tiles x 2 column
            halves) so it can be drip-fed into a later pass's PE slack."""
            state = {}
            thunks = []
            for t in range(4):
                for o0, w in ((0, 512), (512, 256)):
                    def f(t=t, o0=o0, w=w):
                        r0 = 512 * qc + 128 * t
                        if o0 == 0:
                            state[t] = ost.tile([128, DIM], F32, tag="ot",
                                                name=f"ot{qc}_{t}")
                        ot = state[t]
                        po = pr.tile([128, 512], F32, tag="pr")
                        for c in range(NMT):
                            nc.tensor.matmul(po[:, 0:w],
                                             xT[:, c, r0:r0 + 128],
                                             wp_sb[:, c, o0:o0 + w],
                                             start=(c == 0),
                                             stop=(c == NMT - 1))
                        nc.vector.tensor_copy(ot[:, o0:o0 + w], po[:, 0:w])
                        if o0 != 0:
                            nc.sync.dma_start(out=out[r0:r0 + 128, :],
                                              in_=ot[:])
                    thunks.append(f)
            return thunks

        # ---------------- emission schedule ----------------
        kv_chunk_k(0)
        load_w(wq_t, wq_sb, NCT)
        q_chunk(0)
        load_w(wv_t, wv_sb, NCT)
        kv_chunk_v(0)
        load_w(wp_t, wp_sb, NMT)

        # pass (h0, qc0) interleaved with remaining k/v chunks: pair g needs
        # chunk g//2, so emit chunk c at step g=2(c-1) (one-chunk lead)
        def kv_ileave(g):
            if g % 2 == 0:
                c = g // 2 + 1
                if c < NKC:
                    kv_chunk(c)

        def q_ileave(qn):
            def f(g):
                if g == 0:
                    q_chunk(qn)
            return f

        def op_ileave(thunks):
            def f(g):
                if g < len(thunks):
                    thunks[g]()
            return f

        norm = None
        for qc in range(NQC):
            for h in range(NH):
                il = None
                if qc == 0 and h == 0:
                    il = kv_ileave
                elif qc == 0 and h in (1, 2, 3):
                    il = q_ileave(h)
                elif h == 0 and qc > 0:
                    il = op_ileave(outproj_parts(qc - 1))
                # DVE is cast-heavy during the qc0 streaming passes; offload
                # exp steps to it only once the stream has drained
                sch = () if (qc == 0 and h < 4) else (4, 9, 14)
                norm = attn_pass(h, qc, interleave=il, warm=8, pre=norm,
                                 sch_steps=sch)
        norm()
        for f in outproj_parts(NQC - 1):
            f()

    nc.compile()
    return nc


_NC_CACHE = {}


def _get_nc():
    if "nc" not in _NC_CACHE:
        _NC_CACHE["nc"] = build_nc()
    return _NC_CACHE["nc"]


def _core_inputs(query, key, value, Wq, Wk, bk, Wv, Wp):
    """Per-core input dicts: slice batch/query rows and head-dim weights."""
    c = np.ascontiguousarray
    in_maps = []
    for i in range(N_CORES):
        b, hh, qh = i // 4, (i % 4) // 2, i % 2
        sl = slice(HDIM * hh, HDIM * (hh + 1))
        in_maps.append({
            "query": c(query[b, NQ * qh:NQ * (qh + 1), :], dtype=np.float32),
            "key": c(key[b], dtype=np.float32),
            "value": c(value[b], dtype=np.float32),
            "wq_t": c(Wq[sl, :].T, dtype=np.float32),
            "wk_t": c(Wk[sl, :].T, dtype=np.float32),
            "bk_s": c(bk[sl], dtype=np.float32),
            "wv_t": c(Wv[sl, :].T, dtype=np.float32),
            "wp_t": c(Wp[:, sl].T, dtype=np.float32),
        })
    return in_maps


def kernel(query, key, value, Wq, Wk, bk, Wv, Wp, bp, _results_hook=None):
    query = np.asarray(query, dtype=np.float32)
    key = np.asarray(key, dtype=np.float32)
    value = np.asarray(value, dtype=np.float32)
    in_maps = _core_inputs(query, key, value,
                           np.asarray(Wq, dtype=np.float32),
                           np.asarray(Wk, dtype=np.float32),
                           np.asarray(bk, dtype=np.float32),
                           np.asarray(Wv, dtype=np.float32),
                           np.asarray(Wp, dtype=np.float32))
    nc = _get_nc()
    res = bass_utils.run_bass_kernel_spmd(nc, in_maps,
                                          core_ids=list(range(N_CORES)))
    if _results_hook is not None:
        _results_hook(res)

    bp = np.asarray(bp, dtype=np.float32)
    outp = np.empty((B, 2 * NQ, DIM), dtype=np.float32)
    for b in range(B):
        for qh in range(2):
            lo = res.results[b * 4 + qh]["out"]
            hi = res.results[b * 4 + 2 + qh]["out"]
            outp[b, NQ * qh:NQ * (qh + 1), :] = lo + hi + bp
    return outp
